# revision 17
# baseline (speedup 1.0000x reference)
"""VQ codebook-lookup kernel for Trainium2 (8 NeuronCores, data-parallel over batch).

For each (batch, head, token): find nearest codebook row (L2) among 2048 codes,
output that codebook row. argmin ||q - c||^2 == argmax (q.c - 0.5||c||^2).

Per core (one batch of 8):
  - scores computed on TensorE as fp16 main term + fp8 correction terms:
    q.c = qh.ch + ql.c + qh.cl with qh/ch fp16 and the two correction
    terms evaluated as ONE K=256 fp8 DoubleRow matmul (2x rate):
    lhsT = [ql*2^9 (e4m3); qh (e4m3)], rhs = [c*2^-9 (e5m2); cl (e5m2)].
    The 2^9 block scales cancel per-product so everything accumulates in
    the same fp32 PSUM. Score error ~2e-4 -> ~6 argmin flips of 131072
    rows (rel err ~9e-3, under the 2e-2 gate), at 2/3 the tensor time of
    the exact 3-pass fp16 split.
  - fused custom DVE op does bias-add (-0.5||c||^2) + running-max scan +
    argmax-index extraction in ONE 1x pass straight from PSUM
  - GPSIMD indirect DMA gathers the winning codebook rows from DRAM,
    assembled into [128, 1024] out tiles and streamed out during the
    last head's pass
Host side pre-transposes/splits operands (input staging) so no on-chip
transposes are needed.
"""

import numpy as np

import concourse.mybir as mybir
import concourse.tile as tile
from concourse import bacc
from concourse.bass import IndirectOffsetOnAxis
from concourse.bass_utils import run_bass_kernel_spmd

# problem constants (hardcoded per contract)
B = 8  # batch (== n_cores, data-parallel)
N = 2048  # tokens per batch
H = 8  # heads
D = 128  # head dim
M = 2048  # codebook size
NT = N // 128  # 16 n-tiles per head
MB = 4  # m-blocks of 512 per matmul set

f32 = mybir.dt.float32
f16 = mybir.dt.float16
i32 = mybir.dt.int32
bf16 = mybir.dt.bfloat16
f8e4 = mybir.dt.float8e4  # e4m3
f8e5 = mybir.dt.float8e5  # e5m2
FP8_SCALE = 512.0  # 2^9: block scale for the ql/c correction pair

# ---------------------------------------------------------------------------
# custom DVE op: one-pass fused (bias-add, running-max scan, argmax index)
# ---------------------------------------------------------------------------
_ARGMAX_OP = None


def _get_argmax_op():
    global _ARGMAX_OP
    if _ARGMAX_OP is not None:
        return _ARGMAX_OP
    import concourse.dve_ops as dve_ops_mod
    from concourse.dve_ops import CUSTOM_DVE_SPECS, OPS, DveOp
    from concourse.dve_spec import (
        AluOp,
        Idx,
        MaxNeg,
        One,
        Spec,
        Src0,
        Src1,
        Zero,
        eq,
        lower,
        maxx,
        scan,
        select,
    )
    from concourse.dve_uop import DveOpSpec

    name = "ARGMAX_BIAS_ANT"
    for existing in OPS:
        if existing.name == name:  # already registered in this process
            _ARGMAX_OP = existing
            return existing

    def _ref(in0, in1, s0, s1, imm2):
        s = in0.astype(np.float32) + in1.astype(np.float32)
        m = np.maximum.accumulate(s, axis=-1)
        idx = np.arange(s.shape[-1], dtype=np.float32)
        fired = np.where(s == m, idx, -1.0).astype(np.float32)
        acc = fired.max(axis=-1).reshape(s.shape[0], 1).astype(np.float32)
        return fired, acc

    s = Src0 + Src1
    m = scan(AluOp.MAX, s)
    body = select(eq(s, m), Idx, Zero - One)
    spec = Spec(body=body, accum=maxx, accum_init=MaxNeg, reference=_ref)
    shas = {}
    for ver in ("v3", "v4"):
        ups = lower(spec, ver=ver)
        shas[ver] = DveOpSpec(name=name, opcode=0, uops=ups, rd1_en=True).sha(ver)
    op = DveOp(name, spec, subdim=False, uops_sha=shas)
    OPS.append(op)
    CUSTOM_DVE_SPECS[name] = spec
    dve_ops_mod._SUB_OPCODE_FOR_NAME[name] = (
        dve_ops_mod._CUSTOM_DVE_ROW_BASE + len(OPS) - 1
    )
    _ARGMAX_OP = op
    return op


# ---------------------------------------------------------------------------
# bass kernel builder
# ---------------------------------------------------------------------------
_NC_CACHE = None


def _build_nc():
    global _NC_CACHE
    if _NC_CACHE is not None:
        return _NC_CACHE
    argmax_op = _get_argmax_op()

    nc = bacc.Bacc("TRN2", target_bir_lowering=False, debug=False, num_devices=B)

    # DRAM I/O (per-core views; each core gets its own batch slice of q)
    d_qh = nc.dram_tensor("qh", [H, D, N], f16, kind="ExternalInput")
    d_q8 = nc.dram_tensor("q8", [H, D, 2, N], f8e4, kind="ExternalInput")
    d_ch = nc.dram_tensor("ch", [H, D, M], f16, kind="ExternalInput")
    d_c8 = nc.dram_tensor("c8", [H, D, 2, M], f8e5, kind="ExternalInput")
    d_c2 = nc.dram_tensor("c2bc", [H, 128, M], f32, kind="ExternalInput")
    d_cb = nc.dram_tensor("cb", [H * M, D], f32, kind="ExternalInput")
    d_out = nc.dram_tensor("out", [N, H * D], f32, kind="ExternalOutput")

    with tile.TileContext(nc) as tc:
        with (
            tc.tile_pool(name="heads", bufs=2) as hp,
            tc.tile_pool(name="outs", bufs=1) as op_pool,
            tc.tile_pool(name="small", bufs=2) as sp,
            tc.tile_pool(name="scr", bufs=1) as scrp,
            tc.tile_pool(name="ps", bufs=2, space="PSUM") as ps,
        ):
            out_tiles = []
            for t in range(NT):
                ot = op_pool.tile([128, H * D], f32, tag=f"out{t}")
                out_tiles.append(ot)
            scratch = scrp.tile([128, M], bf16, tag="scratch")

            # No HAM warm-up: the kernel is vector-bound, so the first
            # tiles' matmuls ramp the clock themselves without stalling the
            # DVE; warm-up matmuls would block the in-order PE queue while
            # tile-0's DMAs are already done.

            for h in range(H):
                s_qh = hp.tile([D, N], f16, tag="qh")
                s_q8 = hp.tile([D, 2, N], f8e4, tag="q8")
                s_ch = hp.tile([D, M], f16, tag="ch")
                s_c8 = hp.tile([D, 2, M], f8e5, tag="c8")
                s_c2 = hp.tile([128, M], f32, tag="c2")
                # order: tile-0's argmax needs qh/q8 slice 0, full ch/c8/c2
                # (~2.1MB); the remaining qh/q8 slices can land during
                # compute. Spread across both HWDGE rings (sync + scalar).
                if h == 0:
                    # h0 is latency-critical: tile-0's full operand set
                    # (qh/q8 slice 0, ch, c8, c2) lands first across both
                    # rings; the qh/q8 remainders follow in two chunks so
                    # tiles 1-3 unblock before the bulk lands
                    nc.sync.dma_start(s_qh[:, 0:128], d_qh[h][:, 0:128])
                    nc.scalar.dma_start(s_ch[:], d_ch[h])
                    # half of c2 rides the otherwise-idle gpsimd HWDGE ring
                    # so the bias lands in parallel with the matmul operands
                    # (never the vector ring - that queue is the bottleneck)
                    nc.gpsimd.dma_start(s_c2[:, 0:1024], d_c2[h][:, 0:1024])
                    nc.sync.dma_start(s_c8[:], d_c8[h])
                    nc.scalar.dma_start(s_q8[:, :, 0:128], d_q8[h][:, :, 0:128])
                    nc.sync.dma_start(s_c2[:, 1024:1536], d_c2[h][:, 1024:1536])
                    nc.scalar.dma_start(s_c2[:, 1536:], d_c2[h][:, 1536:])
                    nc.sync.dma_start(s_qh[:, 128:512], d_qh[h][:, 128:512])
                    nc.scalar.dma_start(s_q8[:, :, 128:512], d_q8[h][:, :, 128:512])
                    nc.sync.dma_start(s_qh[:, 512:], d_qh[h][:, 512:])
                    nc.scalar.dma_start(s_q8[:, :, 512:], d_q8[h][:, :, 512:])
                else:
                    nc.sync.dma_start(s_qh[:], d_qh[h])
                    nc.scalar.dma_start(s_ch[:], d_ch[h])
                    nc.sync.dma_start(s_c8[:], d_c8[h])
                    nc.scalar.dma_start(s_q8[:], d_q8[h])
                    nc.sync.dma_start(s_c2[:, 0 : M // 2], d_c2[h][:, 0 : M // 2])
                    nc.scalar.dma_start(s_c2[:, M // 2 :], d_c2[h][:, M // 2 :])

                idx_f = sp.tile([128, NT], f32, tag="idxf")
                idx_i = sp.tile([128, NT], i32, tag="idxi")

                for t in range(NT):
                    psc = ps.tile([128, M], f32, tag="scores")
                    qh_t = s_qh[:, t * 128 : (t + 1) * 128]
                    q8_t = s_q8[:, :, t * 128 : (t + 1) * 128]
                    # qh.ch fp16 blocks (start), then the fp8 DoubleRow
                    # correction blocks (K=256: ql.c + qh.cl, stop)
                    for kblk in range(MB):
                        blk = slice(kblk * 512, (kblk + 1) * 512)
                        nc.tensor.matmul(
                            psc[:, blk], qh_t, s_ch[:, blk], start=True, stop=False
                        )
                    for kblk in range(MB):
                        blk = slice(kblk * 512, (kblk + 1) * 512)
                        nc.tensor.matmul(
                            psc[:, blk],
                            q8_t,
                            s_c8[:, :, blk],
                            start=False,
                            stop=True,
                            perf_mode=mybir.MatmulPerfMode.DoubleRow,
                        )
                    # fused bias-add + argmax over m=2048, one DVE pass
                    nc.vector._custom_dve(
                        argmax_op,
                        out=scratch[:],
                        in0=psc[:],
                        in1=s_c2[:],
                        accum_out=idx_f[:, t : t + 1],
                    )
                    # cast f32 index -> i32 on the (idle) scalar engine,
                    # then gather this tile's codebook rows immediately
                    nc.scalar.copy(idx_i[:, t : t + 1], idx_f[:, t : t + 1])
                    nc.gpsimd.indirect_dma_start(
                        out=out_tiles[t][:, h * D : (h + 1) * D],
                        out_offset=None,
                        in_=d_cb[:],
                        in_offset=IndirectOffsetOnAxis(ap=idx_i[:, t : t + 1], axis=0),
                        element_offset=h * M * D,
                    )
                    if h == H - 1:
                        # tile complete after the last head's gather: stream out
                        nc.sync.dma_start(
                            d_out[t * 128 : (t + 1) * 128, :], out_tiles[t][:]
                        )

    nc.compile()
    _NC_CACHE = nc
    return nc


# ---------------------------------------------------------------------------
# host wrapper
# ---------------------------------------------------------------------------


def _prepare_inputs(x, codebooks):
    import concourse.mybir as _mybir

    np_e4 = _mybir.dt.np(f8e4)
    np_e5 = _mybir.dt.np(f8e5)

    x = np.ascontiguousarray(np.asarray(x, dtype=np.float32))
    cb = np.ascontiguousarray(np.asarray(codebooks, dtype=np.float32))

    # q transposed per (batch, head): [B, H, D, N]
    qT = np.ascontiguousarray(x.reshape(B, N, H, D).transpose(0, 2, 3, 1))
    qh = qT.astype(np.float16)
    qlf = qT - qh.astype(np.float32)
    # fp8 correction operands, k-tile packed: q8 = [ql*2^9 ; qh] (e4m3)
    q8 = np.empty((B, H, D, 2, N), dtype=np_e4)
    q8[:, :, :, 0, :] = (qlf * FP8_SCALE).astype(np_e4)
    q8[:, :, :, 1, :] = qh.astype(np.float32).astype(np_e4)

    # codebooks transposed per head: [H, D, M]
    cT = np.ascontiguousarray(cb.transpose(0, 2, 1))
    ch = cT.astype(np.float16)
    clf = cT - ch.astype(np.float32)
    # c8 = [c*2^-9 ; cl] (e5m2)
    c8 = np.empty((H, D, 2, M), dtype=np_e5)
    c8[:, :, 0, :] = (cT / FP8_SCALE).astype(np_e5)
    c8[:, :, 1, :] = clf.astype(np_e5)

    # -0.5 * ||c||^2 broadcast to 128 partitions: [H, 128, M]
    c2 = -0.5 * (cb.astype(np.float64) ** 2).sum(-1)  # [H, M]
    c2bc = np.ascontiguousarray(
        np.broadcast_to(c2.astype(np.float32)[:, None, :], (H, 128, M))
    )

    cb_flat = np.ascontiguousarray(cb.reshape(H * M, D))

    shared = {
        "ch": np.ascontiguousarray(ch),
        "c8": np.ascontiguousarray(c8),
        "c2bc": c2bc,
        "cb": cb_flat,
    }
    in_maps = []
    for b in range(B):
        m = dict(shared)
        m["qh"] = np.ascontiguousarray(qh[b])
        m["q8"] = np.ascontiguousarray(q8[b])
        in_maps.append(m)
    return in_maps


_LAST_RESULTS = None  # stashed for test harness (exec time inspection)


def kernel(x, codebooks, _trace=False, _trace_kwargs=None):
    global _LAST_RESULTS
    import os

    nc = _build_nc()
    in_maps = _prepare_inputs(x, codebooks)
    kw = {}
    if _trace:
        kw["trace"] = True
        kw.update(_trace_kwargs or {})
    else:
        # without the axon NTFF hook installed, a stray BASS_TRACE env would
        # crash run_bass_kernel_spmd on a missing antenv.axon_hooks import
        os.environ["BASS_NEVER_TRACE"] = "1"
    res = run_bass_kernel_spmd(nc, in_maps, core_ids=list(range(B)), **kw)
    if not _trace:
        os.environ.pop("BASS_NEVER_TRACE", None)
    _LAST_RESULTS = res
    out = np.stack([res.results[b]["out"] for b in range(B)], axis=0)
    return out.astype(np.float32)



# revision 18
# speedup vs baseline: 1.0025x; 1.0025x over previous
"""VQ codebook-lookup kernel for Trainium2 (8 NeuronCores, data-parallel over batch).

For each (batch, head, token): find nearest codebook row (L2) among 2048 codes,
output that codebook row. argmin ||q - c||^2 == argmax (q.c - 0.5||c||^2).

Per core (one batch of 8):
  - scores computed on TensorE as fp16 main term + fp8 correction terms:
    q.c = qh.ch + ql.c + qh.cl with qh/ch fp16 and the two correction
    terms evaluated as ONE K=256 fp8 DoubleRow matmul (2x rate):
    lhsT = [ql*2^9 (e4m3); qh (e4m3)], rhs = [c*2^-9 (e5m2); cl (e5m2)].
    The 2^9 block scales cancel per-product so everything accumulates in
    the same fp32 PSUM. Score error ~2e-4 -> ~6 argmin flips of 131072
    rows (rel err ~9e-3, under the 2e-2 gate), at 2/3 the tensor time of
    the exact 3-pass fp16 split.
  - fused custom DVE op does bias-add (-0.5||c||^2) + running-max scan +
    argmax-index extraction in ONE 1x pass straight from PSUM
  - GPSIMD indirect DMA gathers the winning codebook rows from DRAM,
    assembled into [128, 1024] out tiles and streamed out during the
    last head's pass
Host side pre-transposes/splits operands (input staging) so no on-chip
transposes are needed.
"""

import numpy as np

import concourse.mybir as mybir
import concourse.tile as tile
from concourse import bacc
from concourse.bass import IndirectOffsetOnAxis
from concourse.bass_utils import run_bass_kernel_spmd

# problem constants (hardcoded per contract)
B = 8  # batch (== n_cores, data-parallel)
N = 2048  # tokens per batch
H = 8  # heads
D = 128  # head dim
M = 2048  # codebook size
NT = N // 128  # 16 n-tiles per head
MB = 4  # m-blocks of 512 per matmul set

f32 = mybir.dt.float32
f16 = mybir.dt.float16
i32 = mybir.dt.int32
bf16 = mybir.dt.bfloat16
f8e4 = mybir.dt.float8e4  # e4m3
f8e5 = mybir.dt.float8e5  # e5m2
FP8_SCALE = 512.0  # 2^9: block scale for the ql/c correction pair

# ---------------------------------------------------------------------------
# custom DVE op: one-pass fused (bias-add, running-max scan, argmax index)
# ---------------------------------------------------------------------------
_ARGMAX_OP = None


def _get_argmax_op():
    global _ARGMAX_OP
    if _ARGMAX_OP is not None:
        return _ARGMAX_OP
    import concourse.dve_ops as dve_ops_mod
    from concourse.dve_ops import CUSTOM_DVE_SPECS, OPS, DveOp
    from concourse.dve_spec import (
        AluOp,
        Idx,
        MaxNeg,
        One,
        Spec,
        Src0,
        Src1,
        Zero,
        eq,
        lower,
        maxx,
        scan,
        select,
    )
    from concourse.dve_uop import DveOpSpec

    name = "ARGMAX_BIAS_ANT"
    for existing in OPS:
        if existing.name == name:  # already registered in this process
            _ARGMAX_OP = existing
            return existing

    def _ref(in0, in1, s0, s1, imm2):
        s = in0.astype(np.float32) + in1.astype(np.float32)
        m = np.maximum.accumulate(s, axis=-1)
        idx = np.arange(s.shape[-1], dtype=np.float32)
        fired = np.where(s == m, idx, -1.0).astype(np.float32)
        acc = fired.max(axis=-1).reshape(s.shape[0], 1).astype(np.float32)
        return fired, acc

    s = Src0 + Src1
    m = scan(AluOp.MAX, s)
    body = select(eq(s, m), Idx, Zero - One)
    spec = Spec(body=body, accum=maxx, accum_init=MaxNeg, reference=_ref)
    shas = {}
    for ver in ("v3", "v4"):
        ups = lower(spec, ver=ver)
        shas[ver] = DveOpSpec(name=name, opcode=0, uops=ups, rd1_en=True).sha(ver)
    op = DveOp(name, spec, subdim=False, uops_sha=shas)
    OPS.append(op)
    CUSTOM_DVE_SPECS[name] = spec
    dve_ops_mod._SUB_OPCODE_FOR_NAME[name] = (
        dve_ops_mod._CUSTOM_DVE_ROW_BASE + len(OPS) - 1
    )
    _ARGMAX_OP = op
    return op


# ---------------------------------------------------------------------------
# bass kernel builder
# ---------------------------------------------------------------------------
_NC_CACHE = None


def _build_nc():
    global _NC_CACHE
    if _NC_CACHE is not None:
        return _NC_CACHE
    argmax_op = _get_argmax_op()

    nc = bacc.Bacc("TRN2", target_bir_lowering=False, debug=False, num_devices=B)

    # DRAM I/O (per-core views; each core gets its own batch slice of q)
    d_qh = nc.dram_tensor("qh", [H, D, N], f16, kind="ExternalInput")
    d_q8 = nc.dram_tensor("q8", [H, D, 2, N], f8e4, kind="ExternalInput")
    d_ch = nc.dram_tensor("ch", [H, D, M], f16, kind="ExternalInput")
    d_c8 = nc.dram_tensor("c8", [H, D, 2, M], f8e5, kind="ExternalInput")
    d_c2 = nc.dram_tensor("c2bc", [H, 128, M], f32, kind="ExternalInput")
    d_cb = nc.dram_tensor("cb", [H * M, D], f32, kind="ExternalInput")
    d_out = nc.dram_tensor("out", [N, H * D], f32, kind="ExternalOutput")

    with tile.TileContext(nc) as tc:
        with (
            tc.tile_pool(name="heads", bufs=2) as hp,
            tc.tile_pool(name="outs", bufs=1) as op_pool,
            tc.tile_pool(name="small", bufs=2) as sp,
            tc.tile_pool(name="scr", bufs=1) as scrp,
            tc.tile_pool(name="ps", bufs=2, space="PSUM") as ps,
        ):
            out_tiles = []
            for t in range(NT):
                ot = op_pool.tile([128, H * D], f32, tag=f"out{t}")
                out_tiles.append(ot)
            scratch = scrp.tile([128, M], bf16, tag="scratch")

            # No HAM warm-up: the kernel is vector-bound, so the first
            # tiles' matmuls ramp the clock themselves without stalling the
            # DVE; warm-up matmuls would block the in-order PE queue while
            # tile-0's DMAs are already done.

            for h in range(H):
                s_qh = hp.tile([D, N], f16, tag="qh")
                s_q8 = hp.tile([D, 2, N], f8e4, tag="q8")
                s_ch = hp.tile([D, M], f16, tag="ch")
                s_c8 = hp.tile([D, 2, M], f8e5, tag="c8")
                s_c2 = hp.tile([128, M], f32, tag="c2")
                # order: tile-0's argmax needs qh/q8 slice 0, full ch/c8/c2
                # (~2.1MB); the remaining qh/q8 slices can land during
                # compute. Spread across both HWDGE rings (sync + scalar).
                if h == 0:
                    # h0 is latency-critical: tile-0's full operand set
                    # (qh/q8 slice 0, ch, c8, c2) lands first across both
                    # rings; the qh/q8 remainders follow in two chunks so
                    # tiles 1-3 unblock before the bulk lands
                    nc.sync.dma_start(s_qh[:, 0:128], d_qh[h][:, 0:128])
                    nc.scalar.dma_start(s_ch[:, 0:1024], d_ch[h][:, 0:1024])
                    # half of c2 rides the otherwise-idle gpsimd HWDGE ring
                    # so the bias lands in parallel with the matmul operands
                    # (never the vector ring - that queue is the bottleneck)
                    nc.gpsimd.dma_start(s_c2[:, 0:1024], d_c2[h][:, 0:1024])
                    nc.sync.dma_start(s_c8[:], d_c8[h])
                    nc.scalar.dma_start(s_ch[:, 1024:], d_ch[h][:, 1024:])
                    nc.scalar.dma_start(s_q8[:, :, 0:128], d_q8[h][:, :, 0:128])
                    nc.sync.dma_start(s_c2[:, 1024:1536], d_c2[h][:, 1024:1536])
                    nc.scalar.dma_start(s_c2[:, 1536:], d_c2[h][:, 1536:])
                    nc.sync.dma_start(s_qh[:, 128:512], d_qh[h][:, 128:512])
                    nc.scalar.dma_start(s_q8[:, :, 128:512], d_q8[h][:, :, 128:512])
                    nc.sync.dma_start(s_qh[:, 512:], d_qh[h][:, 512:])
                    nc.scalar.dma_start(s_q8[:, :, 512:], d_q8[h][:, :, 512:])
                else:
                    nc.sync.dma_start(s_qh[:], d_qh[h])
                    nc.scalar.dma_start(s_ch[:], d_ch[h])
                    nc.sync.dma_start(s_c8[:], d_c8[h])
                    nc.scalar.dma_start(s_q8[:], d_q8[h])
                    nc.sync.dma_start(s_c2[:, 0 : M // 2], d_c2[h][:, 0 : M // 2])
                    nc.scalar.dma_start(s_c2[:, M // 2 :], d_c2[h][:, M // 2 :])

                idx_f = sp.tile([128, NT], f32, tag="idxf")
                idx_i = sp.tile([128, NT], i32, tag="idxi")

                for t in range(NT):
                    psc = ps.tile([128, M], f32, tag="scores")
                    qh_t = s_qh[:, t * 128 : (t + 1) * 128]
                    q8_t = s_q8[:, :, t * 128 : (t + 1) * 128]
                    # qh.ch fp16 blocks (start), then the fp8 DoubleRow
                    # correction blocks (K=256: ql.c + qh.cl, stop)
                    for kblk in range(MB):
                        blk = slice(kblk * 512, (kblk + 1) * 512)
                        nc.tensor.matmul(
                            psc[:, blk], qh_t, s_ch[:, blk], start=True, stop=False
                        )
                    for kblk in range(MB):
                        blk = slice(kblk * 512, (kblk + 1) * 512)
                        nc.tensor.matmul(
                            psc[:, blk],
                            q8_t,
                            s_c8[:, :, blk],
                            start=False,
                            stop=True,
                            perf_mode=mybir.MatmulPerfMode.DoubleRow,
                        )
                    # fused bias-add + argmax over m=2048, one DVE pass
                    nc.vector._custom_dve(
                        argmax_op,
                        out=scratch[:],
                        in0=psc[:],
                        in1=s_c2[:],
                        accum_out=idx_f[:, t : t + 1],
                    )
                    # cast f32 index -> i32 on the (idle) scalar engine,
                    # then gather this tile's codebook rows immediately
                    nc.scalar.copy(idx_i[:, t : t + 1], idx_f[:, t : t + 1])
                    nc.gpsimd.indirect_dma_start(
                        out=out_tiles[t][:, h * D : (h + 1) * D],
                        out_offset=None,
                        in_=d_cb[:],
                        in_offset=IndirectOffsetOnAxis(ap=idx_i[:, t : t + 1], axis=0),
                        element_offset=h * M * D,
                    )
                    if h == H - 1:
                        # tile complete after the last head's gather: stream out
                        nc.sync.dma_start(
                            d_out[t * 128 : (t + 1) * 128, :], out_tiles[t][:]
                        )

    nc.compile()
    _NC_CACHE = nc
    return nc


# ---------------------------------------------------------------------------
# host wrapper
# ---------------------------------------------------------------------------


def _prepare_inputs(x, codebooks):
    import concourse.mybir as _mybir

    np_e4 = _mybir.dt.np(f8e4)
    np_e5 = _mybir.dt.np(f8e5)

    x = np.ascontiguousarray(np.asarray(x, dtype=np.float32))
    cb = np.ascontiguousarray(np.asarray(codebooks, dtype=np.float32))

    # q transposed per (batch, head): [B, H, D, N]
    qT = np.ascontiguousarray(x.reshape(B, N, H, D).transpose(0, 2, 3, 1))
    qh = qT.astype(np.float16)
    qlf = qT - qh.astype(np.float32)
    # fp8 correction operands, k-tile packed: q8 = [ql*2^9 ; qh] (e4m3)
    q8 = np.empty((B, H, D, 2, N), dtype=np_e4)
    q8[:, :, :, 0, :] = (qlf * FP8_SCALE).astype(np_e4)
    q8[:, :, :, 1, :] = qh.astype(np.float32).astype(np_e4)

    # codebooks transposed per head: [H, D, M]
    cT = np.ascontiguousarray(cb.transpose(0, 2, 1))
    ch = cT.astype(np.float16)
    clf = cT - ch.astype(np.float32)
    # c8 = [c*2^-9 ; cl] (e5m2)
    c8 = np.empty((H, D, 2, M), dtype=np_e5)
    c8[:, :, 0, :] = (cT / FP8_SCALE).astype(np_e5)
    c8[:, :, 1, :] = clf.astype(np_e5)

    # -0.5 * ||c||^2 broadcast to 128 partitions: [H, 128, M]
    c2 = -0.5 * (cb.astype(np.float64) ** 2).sum(-1)  # [H, M]
    c2bc = np.ascontiguousarray(
        np.broadcast_to(c2.astype(np.float32)[:, None, :], (H, 128, M))
    )

    cb_flat = np.ascontiguousarray(cb.reshape(H * M, D))

    shared = {
        "ch": np.ascontiguousarray(ch),
        "c8": np.ascontiguousarray(c8),
        "c2bc": c2bc,
        "cb": cb_flat,
    }
    in_maps = []
    for b in range(B):
        m = dict(shared)
        m["qh"] = np.ascontiguousarray(qh[b])
        m["q8"] = np.ascontiguousarray(q8[b])
        in_maps.append(m)
    return in_maps


_LAST_RESULTS = None  # stashed for test harness (exec time inspection)


def kernel(x, codebooks, _trace=False, _trace_kwargs=None):
    global _LAST_RESULTS
    import os

    nc = _build_nc()
    in_maps = _prepare_inputs(x, codebooks)
    kw = {}
    if _trace:
        kw["trace"] = True
        kw.update(_trace_kwargs or {})
    else:
        # without the axon NTFF hook installed, a stray BASS_TRACE env would
        # crash run_bass_kernel_spmd on a missing antenv.axon_hooks import
        os.environ["BASS_NEVER_TRACE"] = "1"
    res = run_bass_kernel_spmd(nc, in_maps, core_ids=list(range(B)), **kw)
    if not _trace:
        os.environ.pop("BASS_NEVER_TRACE", None)
    _LAST_RESULTS = res
    out = np.stack([res.results[b]["out"] for b in range(B)], axis=0)
    return out.astype(np.float32)



# revision 22
# speedup vs baseline: 1.1814x; 1.1784x over previous
"""VQ codebook-lookup kernel for Trainium2 (8 NeuronCores, data-parallel over batch).

For each (batch, head, token): find nearest codebook row (L2) among 2048 codes,
output that codebook row. argmin ||q - c||^2 == argmax (q.c - 0.5||c||^2).

Per core (one batch of 8):
  - scores computed on TensorE as fp16 main term + fp8 correction terms:
    q.c = qh.ch + ql.c + qh.cl with qh/ch fp16 and the two correction
    terms evaluated as ONE K=256 fp8 DoubleRow matmul (2x rate):
    lhsT = [ql*2^9 (e4m3); qh (e4m3)], rhs = [c*2^-9 (e5m2); cl (e5m2)].
    The 2^9 block scales cancel per-product so everything accumulates in
    the same fp32 PSUM. Score error ~2e-4 -> ~6 argmin flips of 131072
    rows (rel err ~9e-3, under the 2e-2 gate), at 2/3 the tensor time of
    the exact 3-pass fp16 split.
  - fused custom DVE op does bias-add (-0.5||c||^2) + running-max scan +
    argmax-index extraction in ONE 1x pass straight from PSUM
  - GPSIMD indirect DMA gathers the winning codebook rows from DRAM,
    assembled into [128, 1024] out tiles and streamed out during the
    last head's pass
Host side pre-transposes/splits operands (input staging) so no on-chip
transposes are needed.
"""

import numpy as np

import concourse.mybir as mybir
import concourse.tile as tile
from concourse import bacc
from concourse.bass import IndirectOffsetOnAxis
from concourse.bass_utils import run_bass_kernel_spmd

# problem constants (hardcoded per contract)
B = 8  # batch (== n_cores, data-parallel)
N = 2048  # tokens per batch
H = 8  # heads
D = 128  # head dim
M = 2048  # codebook size
NT = N // 128  # 16 n-tiles per head
MB = 4  # m-blocks of 512 per matmul set

f32 = mybir.dt.float32
f16 = mybir.dt.float16
i32 = mybir.dt.int32
bf16 = mybir.dt.bfloat16
f8e4 = mybir.dt.float8e4  # e4m3
f8e5 = mybir.dt.float8e5  # e5m2
FP8_SCALE = 512.0  # 2^9: block scale for the ql/c correction pair

# ---------------------------------------------------------------------------
# custom DVE op: one-pass fused (bias-add, running-max scan, argmax index)
# ---------------------------------------------------------------------------
_ARGMAX_OP = None


def _get_argmax_op():
    global _ARGMAX_OP
    if _ARGMAX_OP is not None:
        return _ARGMAX_OP
    import concourse.dve_ops as dve_ops_mod
    from concourse.dve_ops import CUSTOM_DVE_SPECS, OPS, DveOp
    from concourse.dve_spec import (
        AluOp,
        Idx,
        MaxNeg,
        One,
        Spec,
        Src0,
        Src1,
        Zero,
        eq,
        lower,
        maxx,
        scan,
        select,
    )
    from concourse.dve_uop import DveOpSpec

    name = "ARGMAX_BIAS_ANT"
    for existing in OPS:
        if existing.name == name:  # already registered in this process
            _ARGMAX_OP = existing
            return existing

    def _ref(in0, in1, s0, s1, imm2):
        s = in0.astype(np.float32) + in1.astype(np.float32)
        m = np.maximum.accumulate(s, axis=-1)
        idx = np.arange(s.shape[-1], dtype=np.float32)
        fired = np.where(s == m, idx, -1.0).astype(np.float32)
        acc = fired.max(axis=-1).reshape(s.shape[0], 1).astype(np.float32)
        return fired, acc

    s = Src0 + Src1
    m = scan(AluOp.MAX, s)
    body = select(eq(s, m), Idx, Zero - One)
    spec = Spec(body=body, accum=maxx, accum_init=MaxNeg, reference=_ref)
    shas = {}
    for ver in ("v3", "v4"):
        ups = lower(spec, ver=ver)
        shas[ver] = DveOpSpec(name=name, opcode=0, uops=ups, rd1_en=True).sha(ver)
    op = DveOp(name, spec, subdim=False, uops_sha=shas)
    OPS.append(op)
    CUSTOM_DVE_SPECS[name] = spec
    dve_ops_mod._SUB_OPCODE_FOR_NAME[name] = (
        dve_ops_mod._CUSTOM_DVE_ROW_BASE + len(OPS) - 1
    )
    _ARGMAX_OP = op
    return op


# ---------------------------------------------------------------------------
# bass kernel builder
# ---------------------------------------------------------------------------
_NC_CACHE = None


def _build_nc():
    global _NC_CACHE
    if _NC_CACHE is not None:
        return _NC_CACHE
    argmax_op = _get_argmax_op()

    nc = bacc.Bacc("TRN2", target_bir_lowering=False, debug=False, num_devices=B)

    # DRAM I/O (per-core views; each core gets its own batch slice of q)
    d_qh = nc.dram_tensor("qh", [H, D, N], f16, kind="ExternalInput")
    d_q8 = nc.dram_tensor("q8", [H, D, 2, N], f8e4, kind="ExternalInput")
    d_ch = nc.dram_tensor("ch", [H, D, M], f16, kind="ExternalInput")
    d_c8 = nc.dram_tensor("c8", [H, D, 2, M], f8e5, kind="ExternalInput")
    d_c2 = nc.dram_tensor("c2row", [H, 1, M], f32, kind="ExternalInput")
    d_cb = nc.dram_tensor("cb", [H * M, D], f32, kind="ExternalInput")
    d_out = nc.dram_tensor("out", [N, H * D], f32, kind="ExternalOutput")

    with tile.TileContext(nc) as tc:
        with (
            tc.tile_pool(name="heads", bufs=2) as hp,
            tc.tile_pool(name="outs", bufs=1) as op_pool,
            tc.tile_pool(name="small", bufs=2) as sp,
            tc.tile_pool(name="scr", bufs=1) as scrp,
            tc.tile_pool(name="ps", bufs=2, space="PSUM") as ps,
        ):
            out_tiles = []
            for t in range(NT):
                ot = op_pool.tile([128, H * D], f32, tag=f"out{t}")
                out_tiles.append(ot)
            scratch = scrp.tile([128, M], bf16, tag="scratch")

            # No HAM warm-up: the kernel is vector-bound, so the first
            # tiles' matmuls ramp the clock themselves without stalling the
            # DVE; warm-up matmuls would block the in-order PE queue while
            # tile-0's DMAs are already done.

            for h in range(H):
                s_qh = hp.tile([D, N], f16, tag="qh")
                s_q8 = hp.tile([D, 2, N], f8e4, tag="q8")
                s_ch = hp.tile([D, M], f16, tag="ch")
                s_c8 = hp.tile([D, 2, M], f8e5, tag="c8")
                s_c2 = hp.tile([128, M], f32, tag="c2")
                # order: tile-0's argmax needs qh/q8 slice 0, full ch/c8/c2
                # (~2.1MB); the remaining qh/q8 slices can land during
                # compute. Spread across both HWDGE rings (sync + scalar).
                s_c2r = hp.tile([1, M], f32, tag="c2r")
                if h == 0:
                    # h0 is latency-critical: tile-0's full operand set
                    # (qh/q8 slice 0, ch, c8, c2 row) lands first across
                    # both rings; the qh/q8 remainders follow in two chunks
                    # so tiles 1-3 unblock before the bulk lands
                    nc.sync.dma_start(s_qh[:, 0:128], d_qh[h][:, 0:128])
                    nc.scalar.dma_start(s_c2r[:], d_c2[h])
                    nc.scalar.dma_start(s_ch[:, 0:1024], d_ch[h][:, 0:1024])
                    nc.sync.dma_start(s_c8[:], d_c8[h])
                    nc.scalar.dma_start(s_ch[:, 1024:], d_ch[h][:, 1024:])
                    nc.scalar.dma_start(s_q8[:, :, 0:128], d_q8[h][:, :, 0:128])
                    nc.sync.dma_start(s_qh[:, 128:512], d_qh[h][:, 128:512])
                    nc.scalar.dma_start(s_q8[:, :, 128:512], d_q8[h][:, :, 128:512])
                    nc.sync.dma_start(s_qh[:, 512:], d_qh[h][:, 512:])
                    nc.scalar.dma_start(s_q8[:, :, 512:], d_q8[h][:, :, 512:])
                else:
                    nc.sync.dma_start(s_qh[:], d_qh[h])
                    nc.scalar.dma_start(s_ch[:], d_ch[h])
                    nc.sync.dma_start(s_c8[:], d_c8[h])
                    nc.scalar.dma_start(s_q8[:], d_q8[h])
                    nc.scalar.dma_start(s_c2r[:], d_c2[h])
                # the bias row is broadcast on-chip (gpsimd can address all
                # partitions); saves 1MB/head of HBM traffic and takes the
                # 128x-replicated bias off the latency-critical DMA rings
                nc.gpsimd.partition_broadcast(s_c2[:], s_c2r[:])

                idx_f = sp.tile([128, NT], f32, tag="idxf")
                idx_i = sp.tile([128, NT], i32, tag="idxi")

                for t in range(NT):
                    psc = ps.tile([128, M], f32, tag="scores")
                    qh_t = s_qh[:, t * 128 : (t + 1) * 128]
                    q8_t = s_q8[:, :, t * 128 : (t + 1) * 128]
                    # qh.ch fp16 blocks (start), then the fp8 DoubleRow
                    # correction blocks (K=256: ql.c + qh.cl, stop)
                    for kblk in range(MB):
                        blk = slice(kblk * 512, (kblk + 1) * 512)
                        nc.tensor.matmul(
                            psc[:, blk], qh_t, s_ch[:, blk], start=True, stop=False
                        )
                    for kblk in range(MB):
                        blk = slice(kblk * 512, (kblk + 1) * 512)
                        nc.tensor.matmul(
                            psc[:, blk],
                            q8_t,
                            s_c8[:, :, blk],
                            start=False,
                            stop=True,
                            perf_mode=mybir.MatmulPerfMode.DoubleRow,
                        )
                    # fused bias-add + argmax over m=2048, one DVE pass
                    nc.vector._custom_dve(
                        argmax_op,
                        out=scratch[:],
                        in0=psc[:],
                        in1=s_c2[:],
                        accum_out=idx_f[:, t : t + 1],
                    )
                    # cast f32 index -> i32 on the (idle) scalar engine,
                    # then gather this tile's codebook rows immediately
                    nc.scalar.copy(idx_i[:, t : t + 1], idx_f[:, t : t + 1])
                    nc.gpsimd.indirect_dma_start(
                        out=out_tiles[t][:, h * D : (h + 1) * D],
                        out_offset=None,
                        in_=d_cb[:],
                        in_offset=IndirectOffsetOnAxis(ap=idx_i[:, t : t + 1], axis=0),
                        element_offset=h * M * D,
                    )
                    if h == H - 1:
                        # tile complete after the last head's gather: stream out
                        nc.sync.dma_start(
                            d_out[t * 128 : (t + 1) * 128, :], out_tiles[t][:]
                        )

    nc.compile()
    _NC_CACHE = nc
    return nc


# ---------------------------------------------------------------------------
# host wrapper
# ---------------------------------------------------------------------------


def _prepare_inputs(x, codebooks):
    import concourse.mybir as _mybir

    np_e4 = _mybir.dt.np(f8e4)
    np_e5 = _mybir.dt.np(f8e5)

    x = np.ascontiguousarray(np.asarray(x, dtype=np.float32))
    cb = np.ascontiguousarray(np.asarray(codebooks, dtype=np.float32))

    # q transposed per (batch, head): [B, H, D, N]
    qT = np.ascontiguousarray(x.reshape(B, N, H, D).transpose(0, 2, 3, 1))
    qh = qT.astype(np.float16)
    qlf = qT - qh.astype(np.float32)
    # fp8 correction operands, k-tile packed: q8 = [ql*2^9 ; qh] (e4m3)
    q8 = np.empty((B, H, D, 2, N), dtype=np_e4)
    q8[:, :, :, 0, :] = (qlf * FP8_SCALE).astype(np_e4)
    q8[:, :, :, 1, :] = qh.astype(np.float32).astype(np_e4)

    # codebooks transposed per head: [H, D, M]
    cT = np.ascontiguousarray(cb.transpose(0, 2, 1))
    ch = cT.astype(np.float16)
    clf = cT - ch.astype(np.float32)
    # c8 = [c*2^-9 ; cl] (e5m2)
    c8 = np.empty((H, D, 2, M), dtype=np_e5)
    c8[:, :, 0, :] = (cT / FP8_SCALE).astype(np_e5)
    c8[:, :, 1, :] = clf.astype(np_e5)

    # -0.5 * ||c||^2 as a single row per head (broadcast on-chip): [H, 1, M]
    c2 = -0.5 * (cb.astype(np.float64) ** 2).sum(-1)  # [H, M]
    c2row = np.ascontiguousarray(c2.astype(np.float32)[:, None, :])

    cb_flat = np.ascontiguousarray(cb.reshape(H * M, D))

    shared = {
        "ch": np.ascontiguousarray(ch),
        "c8": np.ascontiguousarray(c8),
        "c2row": c2row,
        "cb": cb_flat,
    }
    in_maps = []
    for b in range(B):
        m = dict(shared)
        m["qh"] = np.ascontiguousarray(qh[b])
        m["q8"] = np.ascontiguousarray(q8[b])
        in_maps.append(m)
    return in_maps


_LAST_RESULTS = None  # stashed for test harness (exec time inspection)


def kernel(x, codebooks, _trace=False, _trace_kwargs=None):
    global _LAST_RESULTS
    import os

    nc = _build_nc()
    in_maps = _prepare_inputs(x, codebooks)
    kw = {}
    if _trace:
        kw["trace"] = True
        kw.update(_trace_kwargs or {})
    else:
        # without the axon NTFF hook installed, a stray BASS_TRACE env would
        # crash run_bass_kernel_spmd on a missing antenv.axon_hooks import
        os.environ["BASS_NEVER_TRACE"] = "1"
    res = run_bass_kernel_spmd(nc, in_maps, core_ids=list(range(B)), **kw)
    if not _trace:
        os.environ.pop("BASS_NEVER_TRACE", None)
    _LAST_RESULTS = res
    out = np.stack([res.results[b]["out"] for b in range(B)], axis=0)
    return out.astype(np.float32)



# revision 26
# speedup vs baseline: 1.3032x; 1.1031x over previous
"""VQ codebook-lookup kernel for Trainium2 (8 NeuronCores, data-parallel over batch).

For each (batch, head, token): find nearest codebook row (L2) among 2048 codes,
output that codebook row. argmin ||q - c||^2 == argmax (q.c - 0.5||c||^2).

Per core (one batch of 8):
  - scores computed on TensorE as fp16 main term + fp8 correction terms:
    q.c = qh.ch + ql.c + qh.cl with qh/ch fp16 and the two correction
    terms evaluated as ONE K=256 fp8 DoubleRow matmul (2x rate):
    lhsT = [ql*2^9 (e4m3); qh (e4m3)], rhs = [c*2^-9 (e5m2); cl (e5m2)].
    The 2^9 block scales cancel per-product so everything accumulates in
    the same fp32 PSUM. Score error ~2e-4 -> ~6 argmin flips of 131072
    rows (rel err ~9e-3, under the 2e-2 gate), at 2/3 the tensor time of
    the exact 3-pass fp16 split.
  - fused custom DVE op does bias-add (-0.5||c||^2) + running-max scan +
    argmax-index extraction in ONE 1x pass straight from PSUM
  - GPSIMD indirect DMA gathers the winning codebook rows from DRAM,
    assembled into [128, 1024] out tiles and streamed out during the
    last head's pass
Host side pre-transposes/splits operands (input staging) so no on-chip
transposes are needed.
"""

import numpy as np

import concourse.mybir as mybir
import concourse.tile as tile
from concourse import bacc
from concourse.bass import IndirectOffsetOnAxis
from concourse.bass_utils import run_bass_kernel_spmd

# problem constants (hardcoded per contract)
B = 8  # batch (== n_cores, data-parallel)
N = 2048  # tokens per batch
H = 8  # heads
D = 128  # head dim
M = 2048  # codebook size
NT = N // 128  # 16 n-tiles per head
MB = 4  # m-blocks of 512 per matmul set

f32 = mybir.dt.float32
f16 = mybir.dt.float16
i32 = mybir.dt.int32
bf16 = mybir.dt.bfloat16
f8e4 = mybir.dt.float8e4  # e4m3
f8e5 = mybir.dt.float8e5  # e5m2
FP8_SCALE = 512.0  # 2^9: block scale for the ql/c correction pair

# ---------------------------------------------------------------------------
# custom DVE op: one-pass fused (bias-add, running-max scan, argmax index)
# ---------------------------------------------------------------------------
_ARGMAX_OP = None


def _get_argmax_op():
    global _ARGMAX_OP
    if _ARGMAX_OP is not None:
        return _ARGMAX_OP
    import concourse.dve_ops as dve_ops_mod
    from concourse.dve_ops import CUSTOM_DVE_SPECS, OPS, DveOp
    from concourse.dve_spec import (
        AluOp,
        Idx,
        MaxNeg,
        One,
        Spec,
        Src0,
        Src1,
        Zero,
        eq,
        lower,
        maxx,
        scan,
        select,
    )
    from concourse.dve_uop import DveOpSpec

    name = "ARGMAX_BIAS_ANT"
    for existing in OPS:
        if existing.name == name:  # already registered in this process
            _ARGMAX_OP = existing
            return existing

    def _ref(in0, in1, s0, s1, imm2):
        s = in0.astype(np.float32) + in1.astype(np.float32)
        m = np.maximum.accumulate(s, axis=-1)
        idx = np.arange(s.shape[-1], dtype=np.float32)
        fired = np.where(s == m, idx, -1.0).astype(np.float32)
        acc = fired.max(axis=-1).reshape(s.shape[0], 1).astype(np.float32)
        return fired, acc

    s = Src0 + Src1
    m = scan(AluOp.MAX, s)
    body = select(eq(s, m), Idx, Zero - One)
    spec = Spec(body=body, accum=maxx, accum_init=MaxNeg, reference=_ref)
    shas = {}
    for ver in ("v3", "v4"):
        ups = lower(spec, ver=ver)
        shas[ver] = DveOpSpec(name=name, opcode=0, uops=ups, rd1_en=True).sha(ver)
    op = DveOp(name, spec, subdim=False, uops_sha=shas)
    OPS.append(op)
    CUSTOM_DVE_SPECS[name] = spec
    dve_ops_mod._SUB_OPCODE_FOR_NAME[name] = (
        dve_ops_mod._CUSTOM_DVE_ROW_BASE + len(OPS) - 1
    )
    _ARGMAX_OP = op
    return op


# ---------------------------------------------------------------------------
# bass kernel builder
# ---------------------------------------------------------------------------
_NC_CACHE = None


def _build_nc():
    global _NC_CACHE
    if _NC_CACHE is not None:
        return _NC_CACHE
    argmax_op = _get_argmax_op()

    nc = bacc.Bacc("TRN2", target_bir_lowering=False, debug=False, num_devices=B)

    # DRAM I/O (per-core views; each core gets its own batch slice of q)
    d_qh = nc.dram_tensor("qh", [H, D, N], f16, kind="ExternalInput")
    d_q8 = nc.dram_tensor("q8", [H, D, 2, N], f8e4, kind="ExternalInput")
    d_ch = nc.dram_tensor("ch", [H, D, M], f16, kind="ExternalInput")
    d_c8 = nc.dram_tensor("c8", [H, D, 2, M], f8e5, kind="ExternalInput")
    d_c2 = nc.dram_tensor("c2row", [H, 1, M], f32, kind="ExternalInput")
    d_cb = nc.dram_tensor("cb", [H * M, D], f32, kind="ExternalInput")
    d_out = nc.dram_tensor("out", [N, H * D], f32, kind="ExternalOutput")

    with tile.TileContext(nc) as tc:
        with (
            tc.tile_pool(name="heads", bufs=2) as hp,
            tc.tile_pool(name="outs", bufs=1) as op_pool,
            tc.tile_pool(name="small", bufs=2) as sp,
            tc.tile_pool(name="scr", bufs=1) as scrp,
            tc.tile_pool(name="ps", bufs=2, space="PSUM") as ps,
        ):
            out_tiles = []
            for t in range(NT):
                ot = op_pool.tile([128, H * D], f32, tag=f"out{t}")
                out_tiles.append(ot)
            # two scratch buffers so consecutive argmaxes don't serialize on
            # a write-after-write semaphore for the throwaway out stream
            scratch0 = scrp.tile([128, M], bf16, tag="scratch0")
            scratch1 = scrp.tile([128, M], bf16, tag="scratch1")
            scratches = [scratch0, scratch1]

            # No HAM warm-up: the kernel is vector-bound, so the first
            # tiles' matmuls ramp the clock themselves without stalling the
            # DVE; warm-up matmuls would block the in-order PE queue while
            # tile-0's DMAs are already done.

            for h in range(H):
                s_qh = hp.tile([D, N], f16, tag="qh")
                s_q8 = hp.tile([D, 2, N], f8e4, tag="q8")
                s_ch = hp.tile([D, M], f16, tag="ch")
                s_c8 = hp.tile([D, 2, M], f8e5, tag="c8")
                s_c2 = hp.tile([128, M], f32, tag="c2")
                # order: tile-0's argmax needs qh/q8 slice 0, full ch/c8/c2
                # (~2.1MB); the remaining qh/q8 slices can land during
                # compute. Spread across both HWDGE rings (sync + scalar).
                s_c2r = hp.tile([1, M], f32, tag="c2r")
                if h == 0:
                    # h0 is latency-critical: tile-0's full operand set
                    # (qh/q8 slice 0, ch, c8, c2 row) lands first across
                    # both rings; the qh/q8 remainders follow in two chunks
                    # so tiles 1-3 unblock before the bulk lands
                    nc.sync.dma_start(s_qh[:, 0:128], d_qh[h][:, 0:128])
                    nc.scalar.dma_start(s_c2r[:], d_c2[h])
                    nc.scalar.dma_start(s_ch[:, 0:1024], d_ch[h][:, 0:1024])
                    nc.sync.dma_start(s_c8[:, :, 0:1024], d_c8[h][:, :, 0:1024])
                    nc.scalar.dma_start(s_q8[:, :, 0:128], d_q8[h][:, :, 0:128])
                    nc.sync.dma_start(s_qh[:, 128:512], d_qh[h][:, 128:512])
                    nc.scalar.dma_start(s_ch[:, 1024:], d_ch[h][:, 1024:])
                    nc.sync.dma_start(s_c8[:, :, 1024:], d_c8[h][:, :, 1024:])
                    nc.scalar.dma_start(s_q8[:, :, 128:512], d_q8[h][:, :, 128:512])
                    nc.sync.dma_start(s_qh[:, 512:], d_qh[h][:, 512:])
                    nc.scalar.dma_start(s_q8[:, :, 512:], d_q8[h][:, :, 512:])
                else:
                    nc.sync.dma_start(s_qh[:], d_qh[h])
                    nc.scalar.dma_start(s_ch[:], d_ch[h])
                    nc.sync.dma_start(s_c8[:], d_c8[h])
                    nc.scalar.dma_start(s_q8[:], d_q8[h])
                    nc.scalar.dma_start(s_c2r[:], d_c2[h])
                # the bias row is broadcast on-chip (gpsimd can address all
                # partitions); saves 1MB/head of HBM traffic and takes the
                # 128x-replicated bias off the latency-critical DMA rings
                nc.gpsimd.partition_broadcast(s_c2[:], s_c2r[:])

                idx_f = sp.tile([128, NT], f32, tag="idxf")
                idx_i = sp.tile([128, NT], i32, tag="idxi")

                for t in range(NT):
                    psc = ps.tile([128, M], f32, tag="scores")
                    qh_t = s_qh[:, t * 128 : (t + 1) * 128]
                    q8_t = s_q8[:, :, t * 128 : (t + 1) * 128]
                    # qh.ch fp16 blocks (start), then the fp8 DoubleRow
                    # correction blocks (K=256: ql.c + qh.cl, stop)
                    for kblk in range(MB):
                        blk = slice(kblk * 512, (kblk + 1) * 512)
                        nc.tensor.matmul(
                            psc[:, blk], qh_t, s_ch[:, blk], start=True, stop=False
                        )
                    for kblk in range(MB):
                        blk = slice(kblk * 512, (kblk + 1) * 512)
                        nc.tensor.matmul(
                            psc[:, blk],
                            q8_t,
                            s_c8[:, :, blk],
                            start=False,
                            stop=True,
                            perf_mode=mybir.MatmulPerfMode.DoubleRow,
                        )
                    # fused bias-add + argmax over m=2048, one DVE pass
                    nc.vector._custom_dve(
                        argmax_op,
                        out=scratches[t % 2][:],
                        in0=psc[:],
                        in1=s_c2[:],
                        accum_out=idx_f[:, t : t + 1],
                    )
                    # cast f32 index -> i32 on the (idle) scalar engine,
                    # then gather this tile's codebook rows immediately
                    nc.scalar.copy(idx_i[:, t : t + 1], idx_f[:, t : t + 1])
                    nc.gpsimd.indirect_dma_start(
                        out=out_tiles[t][:, h * D : (h + 1) * D],
                        out_offset=None,
                        in_=d_cb[:],
                        in_offset=IndirectOffsetOnAxis(ap=idx_i[:, t : t + 1], axis=0),
                        element_offset=h * M * D,
                    )
                    if h == H - 1:
                        # tile complete after the last head's gather: stream out
                        nc.sync.dma_start(
                            d_out[t * 128 : (t + 1) * 128, :], out_tiles[t][:]
                        )

    nc.compile()
    _NC_CACHE = nc
    return nc


# ---------------------------------------------------------------------------
# host wrapper
# ---------------------------------------------------------------------------


def _prepare_inputs(x, codebooks):
    import concourse.mybir as _mybir

    np_e4 = _mybir.dt.np(f8e4)
    np_e5 = _mybir.dt.np(f8e5)

    x = np.ascontiguousarray(np.asarray(x, dtype=np.float32))
    cb = np.ascontiguousarray(np.asarray(codebooks, dtype=np.float32))

    # q transposed per (batch, head): [B, H, D, N]
    qT = np.ascontiguousarray(x.reshape(B, N, H, D).transpose(0, 2, 3, 1))
    qh = qT.astype(np.float16)
    qlf = qT - qh.astype(np.float32)
    # fp8 correction operands, k-tile packed: q8 = [ql*2^9 ; qh] (e4m3)
    q8 = np.empty((B, H, D, 2, N), dtype=np_e4)
    q8[:, :, :, 0, :] = (qlf * FP8_SCALE).astype(np_e4)
    q8[:, :, :, 1, :] = qh.astype(np.float32).astype(np_e4)

    # codebooks transposed per head: [H, D, M]
    cT = np.ascontiguousarray(cb.transpose(0, 2, 1))
    ch = cT.astype(np.float16)
    clf = cT - ch.astype(np.float32)
    # c8 = [c*2^-9 ; cl] (e5m2)
    c8 = np.empty((H, D, 2, M), dtype=np_e5)
    c8[:, :, 0, :] = (cT / FP8_SCALE).astype(np_e5)
    c8[:, :, 1, :] = clf.astype(np_e5)

    # -0.5 * ||c||^2 as a single row per head (broadcast on-chip): [H, 1, M]
    c2 = -0.5 * (cb.astype(np.float64) ** 2).sum(-1)  # [H, M]
    c2row = np.ascontiguousarray(c2.astype(np.float32)[:, None, :])

    cb_flat = np.ascontiguousarray(cb.reshape(H * M, D))

    shared = {
        "ch": np.ascontiguousarray(ch),
        "c8": np.ascontiguousarray(c8),
        "c2row": c2row,
        "cb": cb_flat,
    }
    in_maps = []
    for b in range(B):
        m = dict(shared)
        m["qh"] = np.ascontiguousarray(qh[b])
        m["q8"] = np.ascontiguousarray(q8[b])
        in_maps.append(m)
    return in_maps


_LAST_RESULTS = None  # stashed for test harness (exec time inspection)


def kernel(x, codebooks, _trace=False, _trace_kwargs=None):
    global _LAST_RESULTS
    import os

    nc = _build_nc()
    in_maps = _prepare_inputs(x, codebooks)
    kw = {}
    if _trace:
        kw["trace"] = True
        kw.update(_trace_kwargs or {})
    else:
        # without the axon NTFF hook installed, a stray BASS_TRACE env would
        # crash run_bass_kernel_spmd on a missing antenv.axon_hooks import
        os.environ["BASS_NEVER_TRACE"] = "1"
    res = run_bass_kernel_spmd(nc, in_maps, core_ids=list(range(B)), **kw)
    if not _trace:
        os.environ.pop("BASS_NEVER_TRACE", None)
    _LAST_RESULTS = res
    out = np.stack([res.results[b]["out"] for b in range(B)], axis=0)
    return out.astype(np.float32)

